# revision 1
# baseline (speedup 1.0000x reference)
"""Chamfer loss on 8 trn2 NeuronCores.

Strategy (data-parallel over batch B=8, one batch element per core):
  d[n,m] = ||x_n||^2 + ||y_m||^2 - 2 x_n.y_m  is written as an inner product
  of augmented vectors  u'_n = (-||x_n||^2, -1, 2 x_n),  v_m = (1, ||y_m||^2, y_m)
  so that  u'.v = -d  and the PE computes whole 128x512 tiles of the (negated)
  distance matrix in one matmul.  fp32 accuracy is recovered by splitting each
  augmented vector into bf16 hi/lo limbs stacked along the contraction dim
  (K=20 = 5 components x {uh.vh, uh.vl, ul.vh, ul.vl}), which runs at bf16
  speed (1 cycle/row) instead of fp32's 4 cycles/row.

Engine split (Pool/GpSimd has no streaming ALU on this hardware, ACT has no
two-tensor ops, so ACT drains and DVE reduces):
  ACT   drains every PSUM chunk to bf16 SBUF (plus preamble casts/copies).
  DVE   colmax: running elementwise max of -d across n-tiles (dist2), one
        2x-mode TT per tile; rowmax: fold tree (TT halves at 2x) + batched
        1x reduce every 8 tiles (dist1).
  PE    matmuls + the preamble/tail transposes.
  DMA   tile-0 maxB init, input load, output store.
One shared PSUM pool (tags pb 2x2 banks + pt 4x1 banks) so no pool
release/alloc transition ever gates the pipeline.  Rowmax folds write f3
into a [128, 8, 512] slab finished by three more 2x folds + one small 1x
reduce per 8 tiles; the dist2 tail transposes in 4 groups of 8 blocks.
The kernel outputs per-partition partial sums [128, 2]; the host does the
final 128-way sum, batch mean, and dist1+dist2 add.  Dispatch is an
AOT-compiled fast-dispatch shard_map cached across calls (the baseline
re-jitted every call, costing ~300ms of host time per invocation).
"""
import numpy as np

B, N, M = 8, 4096, 3  # batch, points, coords (N == M == 4096 points per side)
NPTS = 4096

_CACHE = {}


def _patched_tile_context(tile, bass_rust):
    """This walrus build accepts only one sync-wait per instruction; Tile's
    epilogue drain accumulates one wait per processor semaphore.  Split the
    extra waits onto their own SP drain instructions."""

    class PatchedTileContext(tile.TileContext):
        def _drain_and_barrier(self, tick_clock, wait_clock):
            nc = self.nc
            drain_inst = nc.sync.drain()
            wait_clock.add_sem_waits(
                drain_inst.ins, tile.ScopedClock({None: tick_clock.global_clock})
            )
            si = drain_inst.ins.sync_info
            waits = list(si.on_wait) if si is not None else []
            if len(waits) > 1:
                drain_inst.ins.sync_info = bass_rust.SyncInfo(
                    on_wait=[waits[0]], on_update=list(si.on_update)
                )
                for w in waits[1:]:
                    extra = nc.sync.drain()
                    extra.ins.sync_info = bass_rust.SyncInfo(on_wait=[w], on_update=[])
            nc.all_engine_barrier()
            assert self.sems is not None
            popped = nc._tile_sem_poison_stack.pop()
            assert popped is self._sem_poison
            nc.clear_and_free_semaphores(list(self.sems.allocated().values()))
            nc.all_engine_barrier()

    return PatchedTileContext


def _split_multi_waits(nc, mybir, bass_rust):
    """This walrus build accepts only ONE sync-wait per instruction.  Move
    each extra wait onto its own single-wait Drain carrier inserted just
    before the offending instruction (same engine, so program order on that
    engine enforces the wait)."""
    ctr = 0
    for f in nc.m.functions:
        for bb in f.blocks:
            new = []
            for inst in bb.instructions:
                si = getattr(inst, "sync_info", None)
                waits = list(si.on_wait) if si is not None else []
                if len(waits) > 1:
                    for w in waits[:-1]:
                        ctr += 1
                        new.append(
                            bass_rust.InstDrain(
                                name=f"I-wsplit-{ctr}",
                                engine=inst.engine,
                                ins=[],
                                outs=[],
                                sync_info=bass_rust.SyncInfo(
                                    on_wait=[w], on_update=[]
                                ),
                            )
                        )
                    inst.sync_info = bass_rust.SyncInfo(
                        on_wait=[waits[-1]], on_update=list(si.on_update)
                    )
                new.append(inst)
            bb.instructions = new
    return ctr


def _build():
    import bass_rust
    import concourse.bass as bass
    import concourse.mybir as mybir
    import concourse.tile as tile
    from contextlib import ExitStack
    from concourse.masks import make_identity

    F32 = mybir.dt.float32
    BF16 = mybir.dt.bfloat16
    AX = mybir.AxisListType.X
    MAX = mybir.AluOpType.max
    SUB = mybir.AluOpType.subtract

    PatchedTileContext = _patched_tile_context(tile, bass_rust)

    nc = bass.Bass("TRN2", target_bir_lowering=False, debug=False)
    a1 = nc.declare_dram_parameter("array1", [NPTS, 3], F32, isOutput=False)
    a2 = nc.declare_dram_parameter("array2", [NPTS, 3], F32, isOutput=False)
    out_p = nc.declare_dram_parameter("out", [128, 2], F32, isOutput=True)

    with PatchedTileContext(nc) as tc, ExitStack() as ctx:
        singles = ctx.enter_context(tc.tile_pool(name="singles", bufs=1))

        ident = singles.tile([128, 128], BF16)
        make_identity(nc, ident)

        # weight tiles, one per matmul-sized chunk so dep tracking is
        # fine-grained: V pairs [20, 1024] (moving operand needs flat
        # columns), U quads [80, 128] (stationary reads a 20-partition band)
        V20p = [
            singles.tile([20, 1024], BF16, tag=f"v20p{i}", name=f"v20p{i}")
            for i in range(4)
        ]
        U20g = [
            singles.tile([20, 512], BF16, tag=f"u20g{g}", name=f"u20g{g}")
            for g in range(8)
        ]

        def build_w(src, is_u, tag):
            # prep split across engines: pointwise casts/scales on ACT
            # (Square/Copy/mul activations), memsets on Pool, and only the
            # ops that genuinely need DVE (reduce, tensor-tensor subs) there
            # natural layout: point n = 32*p + q on (partition p, slot q)
            eng = nc.vector
            nat = singles.tile([128, 32, 3], F32, tag=f"nat{tag}")
            dma_eng = nc.scalar if is_u else nc.sync  # separate hwdge queues
            dma_eng.dma_start(out=nat, in_=src.rearrange("(p q) d -> p q d", p=128))
            sq = singles.tile([128, 32, 3], F32, tag=f"sq{tag}")
            nc.scalar.square(sq, nat)
            nsq = singles.tile([128, 32, 1], F32, tag=f"nsq{tag}")
            eng.reduce_sum(out=nsq, in_=sq, axis=AX)
            if is_u:
                co = singles.tile([128, 32, 3], F32, tag=f"co{tag}")
                nc.scalar.mul(co, nat, 2.0)
                nsqs = singles.tile([128, 32, 1], F32, tag=f"nsqs{tag}")
                nc.scalar.mul(nsqs, nsq, -1.0)
            else:  # v uses nat / nsq unscaled
                co, nsqs = nat, nsq
            # bf16 hi/lo limb splits (lo = val - upcast(hi), rounded to bf16)
            coh = singles.tile([128, 32, 3], BF16, tag=f"coh{tag}")
            nc.scalar.copy(coh, co)
            cohf = singles.tile([128, 32, 3], F32, tag=f"cohf{tag}")
            nc.scalar.copy(cohf, coh)
            col = singles.tile([128, 32, 3], BF16, tag=f"col{tag}")
            eng.tensor_tensor(out=col, in0=co, in1=cohf, op=SUB)
            nsqh = singles.tile([128, 32, 1], BF16, tag=f"nsqh{tag}")
            nc.scalar.copy(nsqh, nsqs)
            nsqhf = singles.tile([128, 32, 1], F32, tag=f"nsqhf{tag}")
            nc.scalar.copy(nsqhf, nsqh)
            nsql = singles.tile([128, 32, 1], BF16, tag=f"nsql{tag}")
            eng.tensor_tensor(out=nsql, in0=nsqs, in1=nsqhf, op=SUB)

            # K-block layout (contraction dim = 4 limb blocks x 5 slots):
            # U blocks (h, h, l, l), V blocks (h, l, h, l) so the pairwise
            # products cover {hh, hl, lh, ll}.  Adjacent / strided block
            # pairs are written in one broadcast op each.
            W = singles.tile([128, 32, 20], BF16, tag=f"W{tag}")
            nc.gpsimd.memset(W, 0.0)
            W4 = W.rearrange("p q (b k) -> p q b k", b=4)
            hi = W4[:, :, 0:2] if is_u else W4[:, :, 0:4:2]
            lo = W4[:, :, 2:4] if is_u else W4[:, :, 1:4:2]

            def bc(x, k):
                return x.unsqueeze(2).broadcast_to([128, 32, 2, k])

            ceng = nc.scalar if is_u else nc.vector
            if is_u:  # u = (-|x|^2, -1, 2x)
                ceng.copy(hi[:, :, :, 0:1], bc(nsqh, 1))
                nc.gpsimd.memset(hi[:, :, :, 1:2], -1.0)
                ceng.copy(lo[:, :, :, 0:1], bc(nsql, 1))
            else:  # v = (1, |y|^2, y)
                nc.gpsimd.memset(hi[:, :, :, 0:1], 1.0)
                ceng.tensor_copy(hi[:, :, :, 1:2], bc(nsqh, 1))
                ceng.tensor_copy(lo[:, :, :, 1:2], bc(nsql, 1))
            if is_u:
                ceng.copy(hi[:, :, :, 2:5], bc(coh, 3))
                ceng.copy(lo[:, :, :, 2:5], bc(col, 3))
            else:
                ceng.tensor_copy(hi[:, :, :, 2:5], bc(coh, 3))
                ceng.tensor_copy(lo[:, :, :, 2:5], bc(col, 3))
            return W

        # V first: the first matmul chunk needs V pair 0 and U group 0 only.
        # ONE psum pool for everything (preamble transposes, matmul chunks,
        # tail transposes) so no pool release/alloc ever gates progress:
        # tags pb (2 x 2 banks) + pt (4 x 1 bank) = 8 banks.
        Wv = build_w(a2, False, "v")
        Wu = build_w(a1, True, "u")

        # running max of -d over n-tiles (columns = m)
        maxB = singles.tile([128, 4096], BF16)
        dA = singles.tile([128, 32], F32)  # per-row max of -d (col t = n-tile t)

        with tc.tile_pool(name="mm", bufs=1, space="PSUM") as mmp:

            def tgroup_v(i, act_copy=False):  # V pair i = t_idx blocks 8i..8i+7
                pt = mmp.tile([20, 1024], BF16, tag="pt", bufs=4, name=f"ptv{i}")
                for j in range(8):
                    nc.tensor.transpose(
                        pt[:, 128 * j : 128 * (j + 1)], Wv[:, 8 * i + j, :], ident
                    )
                if act_copy:
                    nc.scalar.copy(V20p[i], pt)
                else:
                    nc.vector.tensor_copy(V20p[i], pt)

            def tgroup_u(g):  # U group g = blocks 4g..4g+3, copies on ACT
                pt = mmp.tile([20, 512], BF16, tag="pt", bufs=4, name=f"ptu{g}")
                for j in range(4):
                    nc.tensor.transpose(
                        pt[:, 128 * j : 128 * (j + 1)], Wu[:, 4 * g + j, :], ident
                    )
                nc.scalar.copy(U20g[g], pt)

            # latency-ordered preamble: V pair i feeds tile-0 chunk i, so
            # alternate the copies across ACT (0,2) and DVE (1,3) to let the
            # two drain chains below run in parallel
            tgroup_v(0)
            tgroup_v(1)
            tgroup_u(0)
            tgroup_v(2)
            tgroup_v(3)

            for t in range(32):
                if t % 4 == 0 and t > 0:  # U group t//4 first needed here
                    tgroup_u(t // 4)
                conv = singles.tile(
                    [128, 4096], BF16, tag="conv", bufs=3, name=f"conv{t}"
                )
                ub = U20g[t // 4][:, 128 * (t % 4) : 128 * (t % 4 + 1)]
                for c in range(4):
                    pb = mmp.tile([128, 1024], F32, tag="pb", bufs=2)
                    for j in range(2):
                        s = 2 * c + j
                        nc.tensor.matmul(
                            pb[:, 512 * j : 512 * (j + 1)],
                            ub,
                            V20p[s // 2][:, 512 * (s % 2) : 512 * (s % 2 + 1)],
                            start=True,
                            stop=True,
                        )
                    ch = conv[:, 1024 * c : 1024 * (c + 1)]
                    if t == 0 and c >= 2:  # DVE helps drain tile 0 (it idles)
                        nc.vector.tensor_copy(ch, pb)
                    else:
                        nc.scalar.copy(ch, pb)  # ACT drains PSUM -> bf16 SBUF
                    if t == 0 and c % 2 == 1:  # tile 0: DMA-init maxB
                        nc.sync.dma_start(
                            out=maxB[:, 2048 * (c // 2) : 2048 * (c // 2 + 1)],
                            in_=conv[:, 2048 * (c // 2) : 2048 * (c // 2 + 1)],
                        )
                # colmax: accumulate -d elementwise across n-tiles (DVE 2x)
                if t > 0:
                    nc.vector.tensor_tensor(
                        out=maxB, in0=conv, in1=maxB, op=MAX
                    )
                # rowmax via fold tree (TT-max runs 2x mode, reduce only 1x);
                # the tiny per-tile f4 slabs are batched 4 tiles per reduce
                f1 = singles.tile([128, 2048], BF16, tag="f1", bufs=2, name=f"f1_{t}")
                nc.vector.tensor_tensor(
                    out=f1, in0=conv[:, :2048], in1=conv[:, 2048:], op=MAX
                )
                f2 = singles.tile([128, 1024], BF16, tag="f2", bufs=2, name=f"f2_{t}")
                nc.vector.tensor_tensor(
                    out=f2, in0=f1[:, :1024], in1=f1[:, 1024:], op=MAX
                )
                if t % 8 == 0:
                    f3q = singles.tile(
                        [128, 8, 512], BF16, tag="f3q", bufs=2, name=f"f3q_{t}"
                    )
                nc.vector.tensor_tensor(
                    out=f3q[:, t % 8, :], in0=f2[:, :512], in1=f2[:, 512:], op=MAX
                )
                if t % 8 == 7:  # finish 8 tiles at once: 2x folds then reduce
                    f4q = singles.tile(
                        [128, 8, 256], BF16, tag="f4q", bufs=2, name=f"f4q_{t}"
                    )
                    nc.vector.tensor_tensor(
                        out=f4q, in0=f3q[:, :, :256], in1=f3q[:, :, 256:], op=MAX
                    )
                    f5q = singles.tile(
                        [128, 8, 128], BF16, tag="f5q", bufs=2, name=f"f5q_{t}"
                    )
                    nc.vector.tensor_tensor(
                        out=f5q, in0=f4q[:, :, :128], in1=f4q[:, :, 128:], op=MAX
                    )
                    f6q = singles.tile(
                        [128, 8, 64], BF16, tag="f6q", bufs=2, name=f"f6q_{t}"
                    )
                    nc.vector.tensor_tensor(
                        out=f6q, in0=f5q[:, :, :64], in1=f5q[:, :, 64:], op=MAX
                    )
                    nc.vector.reduce_max(out=dA[:, t - 7 : t + 1], in_=f6q, axis=AX)

            # tail: dist2 = cross-partition max of maxB via PE transposes;
            # per-partition sums are DMA'd out and the host finishes the
            # 128-way reduction.  PSUM tiles come from the SAME pool so the
            # tail needs no pool transition.
            dB = singles.tile([128, 32], F32)
            # dist1 partial sum (dA complete once tile 31's reduce ran)
            dAc = singles.tile([128, 32], F32, name="dAc")
            nc.vector.tensor_scalar_min(dAc, dA, 0.0)
            sAB = singles.tile([128, 2], F32, name="sAB")
            nc.vector.reduce_sum(out=sAB[:, 0:1], in_=dAc, axis=AX)
            for g in range(4):
                pt = mmp.tile([128, 8, 128], BF16, tag="pt", bufs=4, name=f"ptB{g}")
                for j in range(8):
                    i = 8 * g + j
                    nc.tensor.transpose(
                        pt[:, j, :], maxB[:, 128 * i : 128 * (i + 1)], ident
                    )
                nc.vector.reduce_max(out=dB[:, 8 * g : 8 * g + 8], in_=pt, axis=AX)
            # clamp: relu(dist) = -min(max(-d), 0); sum rows per partition
            dBc = singles.tile([128, 32], F32, name="dBc")
            nc.vector.tensor_scalar_min(dBc, dB, 0.0)
            nc.vector.reduce_sum(out=sAB[:, 1:2], in_=dBc, axis=AX)
            nc.sync.dma_start(out=out_p[:], in_=sAB)

    _split_multi_waits(nc, mybir, bass_rust)
    return nc


def _get_compiled():
    """Build the Bass program and AOT-compile the 8-core shard_map once.

    run_bass_via_pjrt re-creates and re-jits its closure on every call
    (full retrace + lowering each time, ~300ms); caching a fast-dispatch
    Compiled object drops warm-call overhead to the actual transfer+exec.
    """
    if "compiled" in _CACHE:
        return _CACHE["compiled"]

    import jax
    import jax.numpy as jnp
    from jax.sharding import Mesh, PartitionSpec
    from concourse import bass2jax as b2j
    from concourse import mybir

    if "nc" not in _CACHE:
        _CACHE["nc"] = _build()
    nc = _CACHE["nc"]

    b2j.install_neuronx_cc_hook()
    assert nc.dbg_addr is None

    partition_name = (
        nc.partition_id_tensor.name if nc.partition_id_tensor else None
    )
    in_names, out_names, out_avals, zero_outs = [], [], [], []
    for alloc in nc.m.functions[0].allocations:
        if not isinstance(alloc, mybir.MemoryLocationSet):
            continue
        name = alloc.memorylocations[0].name
        if alloc.kind == "ExternalInput":
            if name != partition_name:
                in_names.append(name)
        elif alloc.kind == "ExternalOutput":
            shape = tuple(alloc.tensor_shape)
            dtype = mybir.dt.np(alloc.dtype)
            out_names.append(name)
            out_avals.append(jax.core.ShapedArray(shape, dtype))
            zero_outs.append(np.zeros(shape, dtype))
    n_params = len(in_names)
    n_outs = len(out_avals)
    all_in_names = list(in_names) + list(out_names)
    if partition_name is not None:
        all_in_names.append(partition_name)

    def _body(*args):
        operands = list(args)
        if partition_name is not None:
            operands.append(b2j.partition_id_tensor())
        outs = b2j._bass_exec_p.bind(
            *operands,
            out_avals=tuple(out_avals),
            in_names=tuple(all_in_names),
            out_names=tuple(out_names),
            lowering_input_output_aliases=(),
            sim_require_finite=True,
            sim_require_nnan=True,
            nc=nc,
        )
        return tuple(outs)

    devices = jax.devices()[:B]
    assert len(devices) == B
    mesh = Mesh(np.asarray(devices), ("core",))
    in_specs = (PartitionSpec("core"),) * (n_params + n_outs)
    out_specs = (PartitionSpec("core"),) * n_outs
    donate = tuple(range(n_params, n_params + n_outs))

    # example args at the GLOBAL (concat over cores) shape
    ex_in = []
    for name in in_names:
        a = next(
            al
            for al in nc.m.functions[0].allocations
            if isinstance(al, mybir.MemoryLocationSet)
            and al.memorylocations[0].name == name
        )
        shp = tuple(a.tensor_shape)
        ex_in.append(
            jax.ShapeDtypeStruct((B * shp[0],) + shp[1:], mybir.dt.np(a.dtype))
        )
    ex_zero = [
        jax.ShapeDtypeStruct((B * z.shape[0],) + z.shape[1:], z.dtype)
        for z in zero_outs
    ]

    def compile_fn():
        jitted = jax.jit(
            b2j.shard_map(
                _body,
                mesh=mesh,
                in_specs=in_specs,
                out_specs=out_specs,
                check_rep=False,
            ),
            donate_argnums=donate,
            keep_unused=True,
        )
        return jitted.lower(*ex_in, *ex_zero).compile()

    compiled = b2j.fast_dispatch_compile(compile_fn)
    _CACHE["compiled"] = (compiled, in_names, out_names, out_avals, zero_outs)
    return _CACHE["compiled"]


def kernel(array1, array2):
    a1 = np.ascontiguousarray(np.asarray(array1, dtype=np.float32))
    a2 = np.ascontiguousarray(np.asarray(array2, dtype=np.float32))
    assert a1.shape == (B, NPTS, 3) and a2.shape == (B, NPTS, 3)

    compiled, in_names, out_names, out_avals, zero_outs = _get_compiled()

    per_core = {"array1": a1.reshape(B * NPTS, 3), "array2": a2.reshape(B * NPTS, 3)}
    concat_in = [per_core[name] for name in in_names]
    concat_zeros = [
        np.zeros((B * z.shape[0],) + z.shape[1:], z.dtype) for z in zero_outs
    ]
    out_arrs = compiled(*concat_in, *concat_zeros)
    oi = out_names.index("out")
    o = np.asarray(out_arrs[oi], dtype=np.float64).reshape(B, 128, 2)
    tot1 = -o[:, :, 0].sum()  # sum of relu(dist1), per-partition partials
    tot2 = -o[:, :, 1].sum()
    val = tot1 / (B * NPTS) + tot2 / (B * NPTS)
    return np.float32(val)



# revision 4
# speedup vs baseline: 284.3195x; 284.3195x over previous
"""Chamfer loss on 8 trn2 NeuronCores.

Strategy (data-parallel over batch B=8, one batch element per core):
  d[n,m] = ||x_n||^2 + ||y_m||^2 - 2 x_n.y_m  is written as an inner product
  of augmented vectors  u'_n = (-||x_n||^2, -1, 2 x_n),  v_m = (1, ||y_m||^2, y_m)
  so that  u'.v = -d  and the PE computes whole 128x512 tiles of the (negated)
  distance matrix in one matmul.  fp32 accuracy is recovered by splitting each
  augmented vector into bf16 hi/lo limbs stacked along the contraction dim
  (K=20 = 5 components x {uh.vh, uh.vl, ul.vh, ul.vl}), which runs at bf16
  speed (1 cycle/row) instead of fp32's 4 cycles/row.

Engine split (Pool/GpSimd has no streaming ALU on this hardware, ACT has no
two-tensor ops, so ACT drains and DVE reduces):
  ACT   drains each half-tile PSUM slot ([128, 2048] f32 -> bf16 SBUF) in
        ONE activate (amortizes the ~450ns/instr ACT overhead) + preamble.
  DVE   colmax: running elementwise max of -d across n-tiles (dist2), one
        2x-mode TT per tile; rowmax: per-tile f1 fold + per-4-tile f2/f3
        slab folds + per-8-tile f4-f6 + 1x reduce (dist1).
  PE    matmuls + preamble/tail transposes.  ALL of U is transposed in the
        preamble into one [20, 4096] tensor so the steady-state loop has
        zero PE transposes and PSUM is free for 2 x [128, 2048] slots.
  DMA   tile-0 maxB init, input load, output store.
ONE psum tag (pb: 2 bufs x 4 banks = 8 banks); preamble/tail transposes
allocate smaller shapes from the same tag.  Tile 31's colmax is split in
4 column-quarters, each immediately followed by its 8 dist2 transposes +
reduce, so the dist2 tail pipelines instead of serializing at the end.
The kernel outputs per-partition partial sums [128, 2]; the host does the
final 128-way sum, batch mean, and dist1+dist2 add.  Dispatch is an
AOT-compiled fast-dispatch shard_map cached across calls.
"""
import numpy as np

B, N, M = 8, 4096, 3  # batch, points, coords (N == M == 4096 points per side)
NPTS = 4096

_CACHE = {}


def _patched_tile_context(tile, bass_rust):
    """This walrus build accepts only one sync-wait per instruction; Tile's
    epilogue drain accumulates one wait per processor semaphore.  Split the
    extra waits onto their own SP drain instructions."""

    class PatchedTileContext(tile.TileContext):
        def _drain_and_barrier(self, tick_clock, wait_clock):
            nc = self.nc
            drain_inst = nc.sync.drain()
            wait_clock.add_sem_waits(
                drain_inst.ins, tile.ScopedClock({None: tick_clock.global_clock})
            )
            si = drain_inst.ins.sync_info
            waits = list(si.on_wait) if si is not None else []
            if len(waits) > 1:
                drain_inst.ins.sync_info = bass_rust.SyncInfo(
                    on_wait=[waits[0]], on_update=list(si.on_update)
                )
                for w in waits[1:]:
                    extra = nc.sync.drain()
                    extra.ins.sync_info = bass_rust.SyncInfo(on_wait=[w], on_update=[])
            nc.all_engine_barrier()
            assert self.sems is not None
            popped = nc._tile_sem_poison_stack.pop()
            assert popped is self._sem_poison
            nc.clear_and_free_semaphores(list(self.sems.allocated().values()))
            nc.all_engine_barrier()

    return PatchedTileContext


def _split_multi_waits(nc, mybir, bass_rust):
    """This walrus build accepts only ONE sync-wait per instruction.  Move
    each extra wait onto its own single-wait Drain carrier inserted just
    before the offending instruction (same engine, so program order on that
    engine enforces the wait)."""
    ctr = 0
    for f in nc.m.functions:
        for bb in f.blocks:
            new = []
            for inst in bb.instructions:
                si = getattr(inst, "sync_info", None)
                waits = list(si.on_wait) if si is not None else []
                if len(waits) > 1:
                    for w in waits[:-1]:
                        ctr += 1
                        new.append(
                            bass_rust.InstDrain(
                                name=f"I-wsplit-{ctr}",
                                engine=inst.engine,
                                ins=[],
                                outs=[],
                                sync_info=bass_rust.SyncInfo(
                                    on_wait=[w], on_update=[]
                                ),
                            )
                        )
                    inst.sync_info = bass_rust.SyncInfo(
                        on_wait=[waits[-1]], on_update=list(si.on_update)
                    )
                new.append(inst)
            bb.instructions = new
    return ctr


def _build():
    import bass_rust
    import concourse.bass as bass
    import concourse.mybir as mybir
    import concourse.tile as tile
    from contextlib import ExitStack
    from concourse.masks import make_identity

    F32 = mybir.dt.float32
    BF16 = mybir.dt.bfloat16
    AX = mybir.AxisListType.X
    MAX = mybir.AluOpType.max
    SUB = mybir.AluOpType.subtract

    PatchedTileContext = _patched_tile_context(tile, bass_rust)

    nc = bass.Bass("TRN2", target_bir_lowering=False, debug=False)
    a1 = nc.declare_dram_parameter("array1", [NPTS, 3], F32, isOutput=False)
    a2 = nc.declare_dram_parameter("array2", [NPTS, 3], F32, isOutput=False)
    out_p = nc.declare_dram_parameter("out", [128, 2], F32, isOutput=True)

    with PatchedTileContext(nc) as tc, ExitStack() as ctx:
        singles = ctx.enter_context(tc.tile_pool(name="singles", bufs=1))

        ident = singles.tile([128, 128], BF16)
        make_identity(nc, ident)

        # weight tiles: V pairs [20, 1024] (moving operand needs flat
        # columns), U as one [20, 4096] tensor fully transposed in the
        # preamble (no mid-loop PE transposes / PSUM contention)
        V20p = [
            singles.tile([20, 1024], BF16, tag=f"v20p{i}", name=f"v20p{i}")
            for i in range(4)
        ]
        U20 = singles.tile([20, 4096], BF16, tag="u20", name="u20")

        def build_w(src, is_u, tag):
            # prep split across engines: pointwise casts/scales on ACT
            # (Square/Copy/mul activations), memsets on Pool, and only the
            # ops that genuinely need DVE (reduce, tensor-tensor subs) there
            # natural layout: point n = 32*p + q on (partition p, slot q)
            eng = nc.vector
            nat = singles.tile([128, 32, 3], F32, tag=f"nat{tag}")
            dma_eng = nc.scalar if is_u else nc.sync  # separate hwdge queues
            dma_eng.dma_start(out=nat, in_=src.rearrange("(p q) d -> p q d", p=128))
            sq = singles.tile([128, 32, 3], F32, tag=f"sq{tag}")
            nc.scalar.square(sq, nat)
            nsq = singles.tile([128, 32, 1], F32, tag=f"nsq{tag}")
            eng.reduce_sum(out=nsq, in_=sq, axis=AX)
            if is_u:
                co = singles.tile([128, 32, 3], F32, tag=f"co{tag}")
                nc.scalar.mul(co, nat, 2.0)
                nsqs = singles.tile([128, 32, 1], F32, tag=f"nsqs{tag}")
                nc.scalar.mul(nsqs, nsq, -1.0)
            else:  # v uses nat / nsq unscaled
                co, nsqs = nat, nsq
            # bf16 hi/lo limb splits (lo = val - upcast(hi), rounded to bf16)
            coh = singles.tile([128, 32, 3], BF16, tag=f"coh{tag}")
            nc.scalar.copy(coh, co)
            cohf = singles.tile([128, 32, 3], F32, tag=f"cohf{tag}")
            nc.scalar.copy(cohf, coh)
            col = singles.tile([128, 32, 3], BF16, tag=f"col{tag}")
            eng.tensor_tensor(out=col, in0=co, in1=cohf, op=SUB)
            nsqh = singles.tile([128, 32, 1], BF16, tag=f"nsqh{tag}")
            nc.scalar.copy(nsqh, nsqs)
            nsqhf = singles.tile([128, 32, 1], F32, tag=f"nsqhf{tag}")
            nc.scalar.copy(nsqhf, nsqh)
            nsql = singles.tile([128, 32, 1], BF16, tag=f"nsql{tag}")
            eng.tensor_tensor(out=nsql, in0=nsqs, in1=nsqhf, op=SUB)

            # K-block layout (contraction dim = 4 limb blocks x 5 slots):
            # U blocks (h, h, l, l), V blocks (h, l, h, l) so the pairwise
            # products cover {hh, hl, lh, ll}.  Adjacent / strided block
            # pairs are written in one broadcast op each.
            W = singles.tile([128, 32, 20], BF16, tag=f"W{tag}")
            nc.gpsimd.memset(W, 0.0)
            W4 = W.rearrange("p q (b k) -> p q b k", b=4)
            hi = W4[:, :, 0:2] if is_u else W4[:, :, 0:4:2]
            lo = W4[:, :, 2:4] if is_u else W4[:, :, 1:4:2]

            def bc(x, k):
                return x.unsqueeze(2).broadcast_to([128, 32, 2, k])

            ceng = nc.scalar if is_u else nc.vector
            if is_u:  # u = (-|x|^2, -1, 2x)
                ceng.copy(hi[:, :, :, 0:1], bc(nsqh, 1))
                nc.gpsimd.memset(hi[:, :, :, 1:2], -1.0)
                ceng.copy(lo[:, :, :, 0:1], bc(nsql, 1))
            else:  # v = (1, |y|^2, y)
                nc.gpsimd.memset(hi[:, :, :, 0:1], 1.0)
                ceng.tensor_copy(hi[:, :, :, 1:2], bc(nsqh, 1))
                ceng.tensor_copy(lo[:, :, :, 1:2], bc(nsql, 1))
            if is_u:
                ceng.copy(hi[:, :, :, 2:5], bc(coh, 3))
                ceng.copy(lo[:, :, :, 2:5], bc(col, 3))
            else:
                ceng.tensor_copy(hi[:, :, :, 2:5], bc(coh, 3))
                ceng.tensor_copy(lo[:, :, :, 2:5], bc(col, 3))
            return W

        # V first: the first matmul chunk needs V pair 0 and U slot 0 only.
        # ONE psum pool, ONE tag: pb slots are [128, 2048] f32 (4 banks) x 2
        # bufs = all 8 banks.  Preamble/tail transposes allocate smaller
        # shapes from the same tag so they rotate through the same banks.
        Wv = build_w(a2, False, "v")
        Wu = build_w(a1, True, "u")

        # running max of -d over n-tiles (columns = m)
        maxB = singles.tile([128, 4096], BF16)
        dA = singles.tile([128, 32], F32)  # per-row max of -d (col t = n-tile t)
        dB = singles.tile([128, 32], F32)

        with tc.tile_pool(name="mm", bufs=1, space="PSUM") as mmp:

            def tpose_slot(W, i, dst, act_copy, nm):  # blocks 8i..8i+7 of W
                pt = mmp.tile([20, 1024], BF16, tag="pb", bufs=2, name=f"pt{nm}{i}")
                for j in range(8):
                    nc.tensor.transpose(
                        pt[:, 128 * j : 128 * (j + 1)], W[:, 8 * i + j, :], ident
                    )
                if act_copy:
                    nc.scalar.copy(dst, pt)
                else:
                    nc.vector.tensor_copy(dst, pt)

            # latency-ordered preamble: V pair 0 and U slot 0 gate tile 0;
            # the rest overlap the first tiles.  Copies alternate ACT/DVE.
            tpose_slot(Wv, 0, V20p[0], True, "v")
            tpose_slot(Wu, 0, U20[:, 0:1024], False, "u")
            tpose_slot(Wv, 1, V20p[1], True, "v")
            tpose_slot(Wv, 2, V20p[2], False, "v")
            tpose_slot(Wv, 3, V20p[3], True, "v")
            for i in range(1, 4):
                tpose_slot(Wu, i, U20[:, 1024 * i : 1024 * (i + 1)], i % 2, "u")

            for t in range(32):
                conv = singles.tile(
                    [128, 4096], BF16, tag="conv", bufs=4, name=f"conv{t}"
                )
                ub = U20[:, 128 * t : 128 * (t + 1)]
                for h in range(2):  # half-tiles: 4 matmuls + ONE 2048 drain
                    pb = mmp.tile([128, 2048], F32, tag="pb", bufs=2)
                    for j in range(4):
                        s = 4 * h + j
                        nc.tensor.matmul(
                            pb[:, 512 * j : 512 * (j + 1)],
                            ub,
                            V20p[s // 2][:, 512 * (s % 2) : 512 * (s % 2 + 1)],
                            start=True,
                            stop=True,
                        )
                    ch = conv[:, 2048 * h : 2048 * (h + 1)]
                    nc.scalar.copy(ch, pb)  # ACT drains PSUM -> bf16 SBUF
                    if t == 0:  # tile 0: DMA-init maxB
                        nc.sync.dma_start(
                            out=maxB[:, 2048 * h : 2048 * (h + 1)], in_=ch
                        )
                # colmax: accumulate -d elementwise across n-tiles (DVE 2x).
                # Tile 31 is split into quarters so the dist2 transposes +
                # reduces pipeline against the last colmax instead of
                # serializing after it.
                if 0 < t < 31:
                    nc.vector.tensor_tensor(
                        out=maxB, in0=conv, in1=maxB, op=MAX
                    )
                elif t == 31:
                    for q in range(4):
                        mq = maxB[:, 1024 * q : 1024 * (q + 1)]
                        nc.vector.tensor_tensor(
                            out=mq, in0=conv[:, 1024 * q : 1024 * (q + 1)],
                            in1=mq, op=MAX,
                        )
                        ptq = mmp.tile(
                            [128, 8, 128], BF16, tag="pb", bufs=2, name=f"ptq{q}"
                        )
                        for j in range(8):
                            i = 8 * q + j
                            nc.tensor.transpose(
                                ptq[:, j, :], maxB[:, 128 * i : 128 * (i + 1)], ident
                            )
                        nc.vector.reduce_max(
                            out=dB[:, 8 * q : 8 * q + 8], in_=ptq, axis=AX
                        )
                # rowmax fold tree, batched: per tile only f1 runs; f2/f3
                # run once per 4 tiles on [128, 4, *] slabs, f4-f6 + reduce
                # once per 8 tiles (TT-max at 2x mode, reduce only 1x).
                if t % 4 == 0:
                    slabA = singles.tile(
                        [128, 4, 2048], BF16, tag="slabA", bufs=2, name=f"sA{t}"
                    )
                nc.vector.tensor_tensor(
                    out=slabA[:, t % 4, :], in0=conv[:, :2048],
                    in1=conv[:, 2048:], op=MAX,
                )
                if t % 4 == 3:
                    slabB = singles.tile(
                        [128, 4, 1024], BF16, tag="slabB", bufs=2, name=f"sB{t}"
                    )
                    nc.vector.tensor_tensor(
                        out=slabB, in0=slabA[:, :, :1024],
                        in1=slabA[:, :, 1024:], op=MAX,
                    )
                    if t % 8 == 3:
                        slabC = singles.tile(
                            [128, 8, 512], BF16, tag="slabC", bufs=2, name=f"sC{t}"
                        )
                    nc.vector.tensor_tensor(
                        out=slabC[:, 4 * ((t // 4) % 2) : 4 * ((t // 4) % 2) + 4, :],
                        in0=slabB[:, :, :512], in1=slabB[:, :, 512:], op=MAX,
                    )
                if t % 8 == 7:  # finish 8 tiles at once: 2x folds then reduce
                    f4q = singles.tile(
                        [128, 8, 256], BF16, tag="f4q", bufs=2, name=f"f4q_{t}"
                    )
                    nc.vector.tensor_tensor(
                        out=f4q, in0=slabC[:, :, :256], in1=slabC[:, :, 256:], op=MAX
                    )
                    f5q = singles.tile(
                        [128, 8, 128], BF16, tag="f5q", bufs=2, name=f"f5q_{t}"
                    )
                    nc.vector.tensor_tensor(
                        out=f5q, in0=f4q[:, :, :128], in1=f4q[:, :, 128:], op=MAX
                    )
                    f6q = singles.tile(
                        [128, 8, 64], BF16, tag="f6q", bufs=2, name=f"f6q_{t}"
                    )
                    nc.vector.tensor_tensor(
                        out=f6q, in0=f5q[:, :, :64], in1=f5q[:, :, 64:], op=MAX
                    )
                    nc.vector.reduce_max(out=dA[:, t - 7 : t + 1], in_=f6q, axis=AX)

            # tail: only the final sums remain (dist2 transposes already
            # pipelined into tile 31 above).
            dAc = singles.tile([128, 32], F32, name="dAc")
            nc.vector.tensor_scalar_min(dAc, dA, 0.0)
            sAB = singles.tile([128, 2], F32, name="sAB")
            nc.vector.reduce_sum(out=sAB[:, 0:1], in_=dAc, axis=AX)
            # clamp: relu(dist) = -min(max(-d), 0); sum rows per partition
            dBc = singles.tile([128, 32], F32, name="dBc")
            nc.vector.tensor_scalar_min(dBc, dB, 0.0)
            nc.vector.reduce_sum(out=sAB[:, 1:2], in_=dBc, axis=AX)
            nc.sync.dma_start(out=out_p[:], in_=sAB)

    _split_multi_waits(nc, mybir, bass_rust)
    return nc


def _get_compiled():
    """Build the Bass program and AOT-compile the 8-core shard_map once.

    run_bass_via_pjrt re-creates and re-jits its closure on every call
    (full retrace + lowering each time, ~300ms); caching a fast-dispatch
    Compiled object drops warm-call overhead to the actual transfer+exec.
    """
    if "compiled" in _CACHE:
        return _CACHE["compiled"]

    import jax
    import jax.numpy as jnp
    from jax.sharding import Mesh, PartitionSpec
    from concourse import bass2jax as b2j
    from concourse import mybir

    if "nc" not in _CACHE:
        _CACHE["nc"] = _build()
    nc = _CACHE["nc"]

    b2j.install_neuronx_cc_hook()
    assert nc.dbg_addr is None

    partition_name = (
        nc.partition_id_tensor.name if nc.partition_id_tensor else None
    )
    in_names, out_names, out_avals, zero_outs = [], [], [], []
    for alloc in nc.m.functions[0].allocations:
        if not isinstance(alloc, mybir.MemoryLocationSet):
            continue
        name = alloc.memorylocations[0].name
        if alloc.kind == "ExternalInput":
            if name != partition_name:
                in_names.append(name)
        elif alloc.kind == "ExternalOutput":
            shape = tuple(alloc.tensor_shape)
            dtype = mybir.dt.np(alloc.dtype)
            out_names.append(name)
            out_avals.append(jax.core.ShapedArray(shape, dtype))
            zero_outs.append(np.zeros(shape, dtype))
    n_params = len(in_names)
    n_outs = len(out_avals)
    all_in_names = list(in_names) + list(out_names)
    if partition_name is not None:
        all_in_names.append(partition_name)

    def _body(*args):
        operands = list(args)
        if partition_name is not None:
            operands.append(b2j.partition_id_tensor())
        outs = b2j._bass_exec_p.bind(
            *operands,
            out_avals=tuple(out_avals),
            in_names=tuple(all_in_names),
            out_names=tuple(out_names),
            lowering_input_output_aliases=(),
            sim_require_finite=True,
            sim_require_nnan=True,
            nc=nc,
        )
        return tuple(outs)

    devices = jax.devices()[:B]
    assert len(devices) == B
    mesh = Mesh(np.asarray(devices), ("core",))
    in_specs = (PartitionSpec("core"),) * (n_params + n_outs)
    out_specs = (PartitionSpec("core"),) * n_outs
    donate = tuple(range(n_params, n_params + n_outs))

    # example args at the GLOBAL (concat over cores) shape
    ex_in = []
    for name in in_names:
        a = next(
            al
            for al in nc.m.functions[0].allocations
            if isinstance(al, mybir.MemoryLocationSet)
            and al.memorylocations[0].name == name
        )
        shp = tuple(a.tensor_shape)
        ex_in.append(
            jax.ShapeDtypeStruct((B * shp[0],) + shp[1:], mybir.dt.np(a.dtype))
        )
    ex_zero = [
        jax.ShapeDtypeStruct((B * z.shape[0],) + z.shape[1:], z.dtype)
        for z in zero_outs
    ]

    def compile_fn():
        jitted = jax.jit(
            b2j.shard_map(
                _body,
                mesh=mesh,
                in_specs=in_specs,
                out_specs=out_specs,
                check_rep=False,
            ),
            donate_argnums=donate,
            keep_unused=True,
        )
        return jitted.lower(*ex_in, *ex_zero).compile()

    compiled = b2j.fast_dispatch_compile(compile_fn)
    _CACHE["compiled"] = (compiled, in_names, out_names, out_avals, zero_outs)
    return _CACHE["compiled"]


def kernel(array1, array2):
    a1 = np.ascontiguousarray(np.asarray(array1, dtype=np.float32))
    a2 = np.ascontiguousarray(np.asarray(array2, dtype=np.float32))
    assert a1.shape == (B, NPTS, 3) and a2.shape == (B, NPTS, 3)

    compiled, in_names, out_names, out_avals, zero_outs = _get_compiled()

    per_core = {"array1": a1.reshape(B * NPTS, 3), "array2": a2.reshape(B * NPTS, 3)}
    concat_in = [per_core[name] for name in in_names]
    concat_zeros = [
        np.zeros((B * z.shape[0],) + z.shape[1:], z.dtype) for z in zero_outs
    ]
    out_arrs = compiled(*concat_in, *concat_zeros)
    oi = out_names.index("out")
    o = np.asarray(out_arrs[oi], dtype=np.float64).reshape(B, 128, 2)
    tot1 = -o[:, :, 0].sum()  # sum of relu(dist1), per-partition partials
    tot2 = -o[:, :, 1].sum()
    val = tot1 / (B * NPTS) + tot2 / (B * NPTS)
    return np.float32(val)

